# revision 1
# baseline (speedup 1.0000x reference)
"""Cross-attention kernel for Trainium2, SPMD across 8 NeuronCores.

Problem shapes (hardcoded): x [4, 2048, 512], mlp_out [4, 2048, 512],
Wq/Wk/Wv/Wp [512, 512], biases [512]. 8 heads x 64 head-dim.

Sharding: core c handles batch b = c//2 and query rows
[(c%2)*1024 : (c%2+1)*1024).  K/V work is duplicated across the two
cores of a batch pair; in exchange no collective is needed (each core
holds every head for its query rows, so the output projection is local).

Layout strategy per core:
  - Host pre-transposes x/mlp_out/weights so every matmul operand is
    contraction-major (fed as float32r DRAM tensors; the PE reads the
    raw fp32 bits in reduced-precision mode at 1 cycle/row vs 4 for
    fp32, with ~1e-4 relative error).
  - logits are computed transposed ([kj, qi]) so softmax needs no
    max-pass (inputs are small by construction) and the exp output
    feeds the AV matmul with no transposes.
  - The softmax denominator comes from a ones column appended to V, and
    is divided out during the AV eviction (per-partition scalar).

Schedule: the scalar engine's 128 exp instructions (~134us) are the
critical path, so emission order interleaves the projections with the
per-head attention work (exp for heads 2g/2g+1 only needs qT/kT tile g)
and fuses the attn-out transpose + output projection into the last
head's AV loop, keeping ACT busy from ~25us until the end.
"""

import numpy as np

import concourse.bass as bass
import concourse.tile as tile
from concourse import bacc, mybir
from concourse.bass_utils import run_bass_kernel_spmd
from concourse.masks import make_identity

B = 4
N = 2048          # both query and key/value sequence length
C = 512           # model dim
H = 8
D = C // H        # 64
NCORES = 8
QSH = N // 2      # query rows per core (1024)

F32 = mybir.dt.float32
F32R = mybir.dt.float32r
BF16 = mybir.dt.bfloat16


def build_nc(with_bias: bool, reps: int = 1):
    nc = bacc.Bacc("TRN2", target_bir_lowering=False, debug=False)

    xT = nc.dram_tensor("xT", [C, QSH], F32R, kind="ExternalInput")
    mlpT = nc.dram_tensor("mlpT", [C, N], F32R, kind="ExternalInput")
    wqT = nc.dram_tensor("wqT", [C, C], F32R, kind="ExternalInput")
    wkT = nc.dram_tensor("wkT", [C, C], F32R, kind="ExternalInput")
    wvT = nc.dram_tensor("wvT", [C, C], F32R, kind="ExternalInput")
    wpT = nc.dram_tensor("wpT", [C, C], F32R, kind="ExternalInput")
    if with_bias:
        bq = nc.dram_tensor("bq", [1, C], F32, kind="ExternalInput")
        bk = nc.dram_tensor("bk", [1, C], F32, kind="ExternalInput")
        bv = nc.dram_tensor("bv", [1, C], F32, kind="ExternalInput")
        bp = nc.dram_tensor("bp", [1, C], F32, kind="ExternalInput")
    out = nc.dram_tensor("out", [QSH, C], F32, kind="ExternalOutput")

    P = 128
    CT = C // P       # 4 tiles along any model-dim axis
    QT = QSH // P     # 8 query tiles
    KT = N // P       # 16 key tiles
    QB = QSH // 512   # 2 query blocks of 512 (fp32-class moving-dim limit)
    KB = N // 512     # 4

    with tile.TileContext(nc) as tc:
        from contextlib import ExitStack

        with ExitStack() as ctx:
            const = ctx.enter_context(tc.tile_pool(name="const", bufs=1))
            wv_pool = ctx.enter_context(tc.tile_pool(name="wv", bufs=1))
            wp_pool = ctx.enter_context(tc.tile_pool(name="wp", bufs=1))
            sin_pool = ctx.enter_context(tc.tile_pool(name="sin", bufs=8))
            qt_pool = ctx.enter_context(tc.tile_pool(name="qTp", bufs=1))
            kt_pool = ctx.enter_context(tc.tile_pool(name="kTp", bufs=1))
            v_pool = ctx.enter_context(tc.tile_pool(name="vaug", bufs=1))
            ao_pool = ctx.enter_context(tc.tile_pool(name="ao", bufs=1))
            aoT_pool = ctx.enter_context(tc.tile_pool(name="aoT", bufs=2))
            outst = ctx.enter_context(tc.tile_pool(name="outst", bufs=2))
            small = ctx.enter_context(tc.tile_pool(name="small", bufs=8))
            attn_pool = ctx.enter_context(tc.tile_pool(name="attnT", bufs=36))
            proj_ps = ctx.enter_context(
                tc.tile_pool(name="proj_ps", bufs=2, space="PSUM")
            )
            logits_ps = ctx.enter_context(
                tc.tile_pool(name="logits_ps", bufs=2, space="PSUM")
            )
            av_ps = ctx.enter_context(
                tc.tile_pool(name="av_ps", bufs=2, space="PSUM")
            )

            ident = const.tile([P, P], F32)
            make_identity(nc, ident)
            ident_bf = const.tile([P, P], BF16)
            nc.vector.tensor_copy(ident_bf[:], ident[:])

            # optional whole-body repetition for differential wall-clock
            # timing (amortizes host dispatch); reps=1 for production.
            for _rep in range(reps):

              if with_bias:
                  # bias rows as f32r (memset/DMA can't round to f32r; DVE can)
                  def load_row_f32r(dram_row, nm):
                      f = const.tile([1, C], F32, name=f"{nm}_f")
                      nc.sync.dma_start(out=f[:], in_=dram_row[:])
                      r = const.tile([1, C], F32R, name=f"{nm}_r")
                      nc.vector.tensor_copy(r[:], f[:])
                      return r

                  bq_r = load_row_f32r(bq, "bq")
                  bk_r = load_row_f32r(bk, "bk")
                  bv_r = load_row_f32r(bv, "bv")
                  bp_r = load_row_f32r(bp, "bp")
                  ones_f = const.tile([1, 512], F32)
                  nc.vector.memset(ones_f[:], 1.0)
                  ones_r = const.tile([1, 512], F32R)
                  nc.vector.tensor_copy(ones_r[:], ones_f[:])

              # ---- weight tiles (resident) ----
              def make_tiles(pool, cols, pref, n=CT):
                  return [
                      pool.tile([P, cols], F32R, tag=f"{pref}{ct}",
                                name=f"{pref}{ct}")
                      for ct in range(n)
                  ]

              wqt = [attn_pool.tile([P, C], F32R, tag="attnT", name=f"wq{ct}")
                     for ct in range(CT)]
              wkt = [attn_pool.tile([P, C], F32R, tag="attnT", name=f"wk{ct}")
                     for ct in range(CT)]
              wvt = make_tiles(wv_pool, C, "wv")
              wpt = make_tiles(wp_pool, C, "wp")

              def dma_rows(tiles, dram, col0, col1):
                  for ct, t in enumerate(tiles):
                      nc.sync.dma_start(
                          out=t[:, col0:col1],
                          in_=dram[ct * P : (ct + 1) * P, col0:col1],
                      )

              dma_rows(wqt, wqT, 0, C)

              def load_colblock(dram, col0, pref):
                  # one [512, 512] contraction-major column block as 4 tiles
                  ts = []
                  for cc in range(CT):
                      t = sin_pool.tile([P, 512], F32R, tag="sin",
                                        name=f"{pref}{cc}")
                      nc.sync.dma_start(
                          out=t[:], in_=dram[cc * P : (cc + 1) * P,
                                             col0 : col0 + 512]
                      )
                      ts.append(t)
                  return ts

              qT = [qt_pool.tile([P, QSH], F32R, tag=f"qT{i}", name=f"qT{i}")
                    for i in range(CT)]
              kT = [kt_pool.tile([P, N], F32R, tag=f"kT{i}", name=f"kT{i}")
                    for i in range(CT)]
              vaug = [v_pool.tile([P, H, D + 1], BF16, tag=f"v{i}",
                                  name=f"vaug{i}") for i in range(KT)]
              attn_out = [ao_pool.tile([P, C], BF16, tag=f"ao{i}", name=f"ao{i}")
                          for i in range(QT)]

              def proj_qT(mt, qb, xblk):
                  ps = proj_ps.tile([P, 512], F32, tag="proj", name="ps_q")
                  for cc in range(CT):
                      nc.tensor.matmul(
                          ps[:],
                          wqt[cc][:, mt * P : (mt + 1) * P],
                          xblk[cc][:],
                          start=(cc == 0),
                          stop=(cc == CT - 1 and not with_bias),
                      )
                  if with_bias:
                      nc.tensor.matmul(
                          ps[:], bq_r[:, mt * P : (mt + 1) * P], ones_r[:],
                          start=False, stop=True,
                      )
                  nc.vector.tensor_copy(qT[mt][:, qb * 512 : (qb + 1) * 512],
                                        ps[:])

              def proj_kT(mt, kb, mblk):
                  ps = proj_ps.tile([P, 512], F32, tag="proj", name="ps_k")
                  for cc in range(CT):
                      nc.tensor.matmul(
                          ps[:],
                          wkt[cc][:, mt * P : (mt + 1) * P],
                          mblk[cc][:],
                          start=(cc == 0),
                          stop=(cc == CT - 1 and not with_bias),
                      )
                  if with_bias:
                      nc.tensor.matmul(
                          ps[:], bk_r[:, mt * P : (mt + 1) * P], ones_r[:],
                          start=False, stop=True,
                      )
                  nc.vector.tensor_copy(kT[mt][:, kb * 512 : (kb + 1) * 512],
                                        ps[:])

              def proj_v(kt, mblk):
                  # kt is global; mblk holds columns [ (kt//4)*512 , +512 )
                  # Uses the av PSUM pool (same bank footprint) so the deferred
                  # V stream never contends with the kT projections' PSUM.
                  lo = (kt % 4) * P
                  ps = av_ps.tile([P, 512], F32, tag="av", name="ps_v")
                  for cc in range(CT):
                      nc.tensor.matmul(
                          ps[:],
                          mblk[cc][:, lo : lo + P],
                          wvt[cc][:],
                          start=(cc == 0),
                          stop=(cc == CT - 1 and not with_bias),
                      )
                  if with_bias:
                      ones_col = small.tile([1, P], F32R, tag="onec", name="onec")
                      nc.vector.tensor_copy(ones_col[:], ones_r[:, 0:P])
                      nc.tensor.matmul(ps[:], ones_col[:], bv_r[:],
                                       start=False, stop=True)
                  nc.vector.tensor_copy(
                      vaug[kt][:, :, 0:D],
                      ps[:].rearrange("p (h d) -> p h d", h=H),
                  )
                  nc.vector.memset(vaug[kt][:, :, D : D + 1], 1.0)

              def head_qk_exp(h, kts, tiles):
                  mt, po = h // 2, (h % 2) * D
                  for kt in kts:
                      lp = logits_ps.tile([P, QSH], F32, tag="logits", name="lp")
                      for qb in range(QB):
                          nc.tensor.matmul(
                              lp[:, qb * 512 : (qb + 1) * 512],
                              kT[mt][po : po + D, kt * P : (kt + 1) * P],
                              qT[mt][po : po + D, qb * 512 : (qb + 1) * 512],
                              start=True,
                              stop=True,
                          )
                      at = attn_pool.tile([P, QSH], BF16, tag="attnT", name="at")
                      nc.scalar.activation(
                          out=at[:], in_=lp[:],
                          func=mybir.ActivationFunctionType.Exp,
                      )
                      tiles.append(at)
                  return tiles

              def head_av(h, attnT_h, qt):
                  av = av_ps.tile([P, D + 1], F32, tag="av", name="av")
                  for kt in range(KT):
                      nc.tensor.matmul(
                          av[:],
                          attnT_h[kt][:, qt * P : (qt + 1) * P],
                          vaug[kt][:, h, :],
                          start=(kt == 0),
                          stop=(kt == KT - 1),
                      )
                  recip = small.tile([P, 1], F32, tag="recip", name="recip")
                  nc.vector.reciprocal(recip[:], av[:, D : D + 1])
                  nc.vector.tensor_scalar_mul(
                      attn_out[qt][:, h * D : (h + 1) * D], av[:, 0:D], recip[:]
                  )

              def tail_qt(qt):
                  # transpose attn_out[qt] (dh-major blocks), then project
                  ps = proj_ps.tile([P, 512], BF16, tag="proj", name="ps_t")
                  for mt in range(CT):
                      nc.tensor.transpose(
                          ps[:, mt * P : (mt + 1) * P],
                          attn_out[qt][:, mt * P : (mt + 1) * P],
                          ident_bf[:],
                      )
                  aoTq = aoT_pool.tile([P, CT, P], F32R, tag="aoTq", name="aoTq")
                  nc.vector.tensor_copy(
                      aoTq[:], ps[:].rearrange("p (m q) -> p m q", m=CT)
                  )
                  po = proj_ps.tile([P, 512], F32, tag="proj", name="ps_o")
                  for mt in range(CT):
                      nc.tensor.matmul(
                          po[:],
                          aoTq[:, mt, :],
                          wpt[mt][:],
                          start=(mt == 0),
                          stop=(mt == CT - 1 and not with_bias),
                      )
                  if with_bias:
                      ones_col = small.tile([1, P], F32R, tag="onec", name="onec")
                      nc.vector.tensor_copy(ones_col[:], ones_r[:, 0:P])
                      nc.tensor.matmul(po[:], ones_col[:], bp_r[:],
                                       start=False, stop=True)
                  o = outst.tile([P, C], F32, tag="outst", name="outst")
                  nc.vector.tensor_copy(o[:], po[:])
                  nc.sync.dma_start(out=out[qt * P : (qt + 1) * P, :], in_=o[:])

              # ---- streamed projections, heads 0/1 interleaved per kb chunk.
              # attnT has 24 slots; at any moment the previous head's 16 tiles
              # plus at most 8 of the next head's may be live, so each head's
              # last 8 exp chunks are emitted only after the previous head's
              # AV (which releases its 16 slots).
              for qb in range(QB):
                  xblk = load_colblock(xT, qb * 512, f"xb{qb}_")
                  for mt in range(CT):
                      proj_qT(mt, qb, xblk)
              dma_rows(wkt, wkT, 0, C)
              tiles = {0: [], 1: []}
              vblks = []
              for kb in range(KB):
                  mblk = load_colblock(mlpT, kb * 512, f"mb{kb}_")
                  if kb == 0:
                      dma_rows(wvt, wvT, 0, C)
                  else:
                      # second-pass stream for the deferred V projections,
                      # interleaved into the DMA queue so it lands early
                      vblks.append(load_colblock(mlpT, (kb - 1) * 512,
                                                 f"vb{kb-1}_"))
                  proj_kT(0, kb, mblk)
                  head_qk_exp(0, range(4 * kb, 4 * kb + 4), tiles[0])
                  if kb >= 1:  # h1's first 12 chunks ride along in the loop
                      head_qk_exp(1, range(4 * (kb - 1), 4 * (kb - 1) + 4),
                                  tiles[1])
                  proj_kT(1, kb, mblk)
              vblks.append(load_colblock(mlpT, (KB - 1) * 512, f"vb{KB-1}_"))
              dma_rows(wpt, wpT, 0, C)
              for kb in range(KB):
                  for r in range(4):
                      proj_v(kb * 4 + r, vblks[kb])

              # kT for head pairs 2/3 (heads 4-7) deferred out of the window:
              # third mlp stream pass while the DMA engines are otherwise idle.
              for kb in range(KB):
                  kblk = load_colblock(mlpT, kb * 512, f"kb2_{kb}_")
                  proj_kT(2, kb, kblk)
                  proj_kT(3, kb, kblk)

              for h in range(H):
                  nxt = h + 1
                  if nxt < H:
                      t_n = tiles.setdefault(nxt, [])
                      head_qk_exp(nxt, range(len(t_n), KT), t_n)
                  for qt in range(QT):
                      head_av(h, tiles[h], qt)
                      if h == H - 1:
                          tail_qt(qt)

    nc.compile()
    return nc


_CACHE: dict = {}


def get_nc(with_bias: bool):
    key = ("nc", with_bias)
    if key not in _CACHE:
        _CACHE[key] = build_nc(with_bias)
    return _CACHE[key]


def make_in_maps(inputs: dict) -> tuple[list[dict], bool]:
    x = np.asarray(inputs["x"], dtype=np.float32)
    mlp = np.asarray(inputs["mlp_out"], dtype=np.float32)
    Wq = np.asarray(inputs["Wq"], dtype=np.float32)
    Wk = np.asarray(inputs["Wk"], dtype=np.float32)
    Wv = np.asarray(inputs["Wv"], dtype=np.float32)
    Wp = np.asarray(inputs["Wp"], dtype=np.float32)
    bq = np.asarray(inputs["bq"], dtype=np.float32)
    bk = np.asarray(inputs["bk"], dtype=np.float32)
    bv = np.asarray(inputs["bv"], dtype=np.float32)
    bp = np.asarray(inputs["bp"], dtype=np.float32)

    with_bias = bool(np.any(bq) or np.any(bk) or np.any(bv) or np.any(bp))

    wqT = np.ascontiguousarray(Wq.T)  # [c, dh]
    wkT = np.ascontiguousarray(Wk.T)
    wvT = np.ascontiguousarray(Wv.T)
    wpT = np.ascontiguousarray(Wp.T)  # [dh, co]

    in_maps = []
    for c in range(NCORES):
        b, half = c // 2, c % 2
        xs = np.ascontiguousarray(x[b, half * QSH : (half + 1) * QSH, :].T)
        ms = np.ascontiguousarray(mlp[b].T)
        m = {
            "xT": xs, "mlpT": ms,
            "wqT": wqT, "wkT": wkT, "wvT": wvT, "wpT": wpT,
        }
        if with_bias:
            m["bq"] = bq.reshape(1, C)
            m["bk"] = bk.reshape(1, C)
            m["bv"] = bv.reshape(1, C)
            m["bp"] = bp.reshape(1, C)
        in_maps.append(m)
    return in_maps, with_bias


def kernel(**inputs) -> np.ndarray:
    in_maps, with_bias = make_in_maps(inputs)
    nc = get_nc(with_bias)
    res = run_bass_kernel_spmd(nc, in_maps, list(range(NCORES)))
    full = np.empty((B, N, C), dtype=np.float32)
    for c in range(NCORES):
        b, half = c // 2, c % 2
        full[b, half * QSH : (half + 1) * QSH, :] = res.results[c]["out"]
    return full

